# revision 26
# baseline (speedup 1.0000x reference)
"""Fused pairwise-MLP kernel for Trainium2 (8 NeuronCores, SPMD data-parallel).

Computes log_q[i, j] = W3 @ gelu(W2 @ gelu(a[j] + b[i] + b1) + b2) + b3
with a = z1 @ W1a.T, b = z2 @ W1b.T  (W1 = [W1a | W1b]), N=1024, H=EMB=128.

Sharding: rows of i (z2) split across 8 cores, z1 + weights replicated
(host-side sharding; no collectives). The [N, N, H] intermediates never
touch HBM.

v2: gelu1 is split across TWO engines. Most i-rows run on the DVE via a
custom 8-slice fused op (GELU_QS_ANT): out = xt*(1 + xt*(1 - c*|xt|)) with
xt = a_fp16 + bb_i (the per-i bias add is fused via the per-partition C0/C1
scalar slots; |xt| = max(xt, -xt) built from C1 = -bb_i). In x-space this is
4*q1*x*(0.5 + q1*x - q2*x*|x|), a quadratic-sigmoid gelu approximation
(fitted; end-to-end rel err ~4e-3 << 2e-2 gate). The 1/(4*q1) and lambda
factors fold into a second fp16 W2 copy used only for DVE-produced rows.
The remaining rows run exact gelu on ACT via the bias port (host pre-scales
W1/b1 by s=2*q1; ACT undoes it with scale=1/s). ACT otherwise does gelu2
(1536-wide from the 2x1536 PSUM ring). W3 matmuls accumulate all 128 output
rows into a dedicated 2-bank PSUM strip (row = local i via 32 single-column
W3 variants + tile_position col groups); output leaves via 4 chunked
DVE +b3 evacuations and 8 spread DMAs.
"""

import numpy as np

import concourse.bacc as bacc
import concourse.bass as bass
import concourse.tile as tile
import concourse.mybir as mybir
from concourse import bass_utils
from concourse import dve_ops
from concourse.dve_spec import (
    Spec, Src0, C0, C1, C2, One, maxx, lower, Bin, AluOp,
)
from concourse.dve_uop import DveOpSpec


N = 1024
EMB = 128
HID = 128
NCORES = 8
SH = N // NCORES  # i-rows per core
F32 = mybir.dt.float32
FP16 = mybir.dt.float16
GELU = mybir.ActivationFunctionType.Gelu

# quadratic-sigmoid gelu fit (see docstring): gelu(x) ~= LAM*x*(0.5+Q1*x-Q2*x*|x|)
Q1 = 0.43679
Q2 = 0.09786
LAM = 1.00001
S_Y = 2.0 * Q1                 # host pre-scale of W1/b1
C_T = Q2 / (2.0 * Q1 * Q1)     # |.| coefficient in y-space
WDVE = LAM / (4.0 * Q1)        # folded into the DVE-row W2 copy

# ACT rows per 16-row block (rest go to the DVE custom op)
K_PATTERN = (2, 2, 2, 2, 2, 2, 2, 1)


def _is_act_row(il: int) -> bool:
    return (il % 16) < K_PATTERN[il // 16]


def _register_gelu_op():
    name = "GELU_QS_ANT"
    for op in dve_ops.OPS:
        if op.name == name:
            return op
    xt = Src0 + C0
    neg = Bin(AluOp.SUBTRACT, C1, Src0)      # C1 = -bb  ->  -xt
    u = maxx(xt, neg)                        # |xt|
    w2 = Bin(AluOp.SUBTRACT, One, u * C2)    # 1 - c*|xt|
    body = (w2 * xt + One) * xt              # xt*(1 + xt*(1 - c*|xt|))

    def _ref(in0, in1, s0, s1, imm2):
        x = in0.astype(np.float32) + s0
        return (x * (1.0 + x * (1.0 - imm2 * np.abs(x)))).astype(np.float32)

    spec = Spec(body=body, reference=_ref)
    row = dve_ops._CUSTOM_DVE_ROW_BASE + len(dve_ops.OPS)
    assert row < 0x20
    shas = {}
    for ver in ("v3", "v4"):
        s = DveOpSpec(name=name, opcode=row, uops=lower(spec, ver=ver), rd1_en=False)
        shas[ver] = s.sha(ver)
    op = dve_ops.DveOp(name, spec, subdim=False, uops_sha=shas)
    dve_ops.OPS.append(op)
    dve_ops._SUB_OPCODE_FOR_NAME[name] = row
    dve_ops.CUSTOM_DVE_SPECS[name] = spec
    return op


GELU_OP = _register_gelu_op()


def _build():
    nc = bacc.Bacc("TRN2", target_bir_lowering=False, debug=False)

    z1c_d = [
        nc.dram_tensor(f"z1c{c}", (EMB, 128), F32, kind="ExternalInput")
        for c in range(8)
    ]
    z2T_d = nc.dram_tensor("z2T", (EMB, SH), F32, kind="ExternalInput")
    w1aT_d = nc.dram_tensor("w1aT", (EMB, HID), F32, kind="ExternalInput")
    w1bT_d = nc.dram_tensor("w1bT", (EMB, HID), F32, kind="ExternalInput")
    b1y_d = nc.dram_tensor("b1y", (HID, 1), F32, kind="ExternalInput")
    b2c_d = nc.dram_tensor("b2c", (HID, 1), F32, kind="ExternalInput")
    b3c_d = nc.dram_tensor("b3c", (HID, 1), F32, kind="ExternalInput")
    w2T_d = nc.dram_tensor("w2T", (HID, HID), FP16, kind="ExternalInput")
    w2Td_d = nc.dram_tensor("w2Td", (HID, HID), FP16, kind="ExternalInput")
    w3bank_d = nc.dram_tensor("w3bank", (HID, 32 * 32), FP16, kind="ExternalInput")
    out_d = nc.dram_tensor("out", (SH, N), F32, kind="ExternalOutput")

    with tile.TileContext(nc) as tc:
        _body(tc, out_d, z1c_d, z2T_d, w1aT_d, w1bT_d, b1y_d, b2c_d, b3c_d,
              w2T_d, w2Td_d, w3bank_d)

    nc.compile()
    return nc


def _body(tc, out_d, z1c_d, z2T_d, w1aT_d, w1bT_d, b1y_d, b2c_d, b3c_d,
          w2T_d, w2Td_d, w3bank_d):
    nc = tc.nc
    with (
        tc.tile_pool(name="const", bufs=1) as const,
        tc.tile_pool(name="h1p", bufs=4) as h1p,
        tc.tile_pool(name="h2p", bufs=6) as h2p,
        tc.tile_pool(name="ringp", bufs=1, space="PSUM") as ringp,
        tc.tile_pool(name="stripp", bufs=1, space="PSUM") as stripp,
    ):
        # ---- input DMAs, spread across the engine HWDGE queues ----
        z2T_sb = const.tile([128, SH], F32)
        nc.sync.dma_start(out=z2T_sb, in_=z2T_d.ap())
        w1bT_sb = const.tile([128, HID], F32)
        nc.sync.dma_start(out=w1bT_sb, in_=w1bT_d.ap())
        w1aT_sb = const.tile([128, HID], F32)
        nc.scalar.dma_start(out=w1aT_sb, in_=w1aT_d.ap())

        z1T_sb = const.tile([128, N], F32)
        for c, eng in enumerate((nc.sync, nc.scalar, nc.gpsimd, nc.sync,
                                 nc.scalar, nc.gpsimd, nc.sync, nc.scalar)):
            eng.dma_start(out=z1T_sb[:, c * 128:(c + 1) * 128], in_=z1c_d[c].ap())

        b1y_sb = const.tile([128, 1], F32)
        nc.gpsimd.dma_start(out=b1y_sb, in_=b1y_d.ap())
        b2c_sb = const.tile([128, 1], F32)
        nc.gpsimd.dma_start(out=b2c_sb, in_=b2c_d.ap())
        b3c_sb = const.tile([128, 1], F32)
        nc.gpsimd.dma_start(out=b3c_sb, in_=b3c_d.ap())
        w2T_sb = const.tile([128, HID], FP16)
        nc.gpsimd.dma_start(out=w2T_sb, in_=w2T_d.ap())
        w2Td_sb = const.tile([128, HID], FP16)
        nc.gpsimd.dma_start(out=w2Td_sb, in_=w2Td_d.ap())
        w3bank_sb = const.tile([128, 32 * 32], FP16)
        nc.gpsimd.dma_start(out=w3bank_sb, in_=w3bank_d.ap())

        # warm the gelu ACT table off the critical path
        tiny = const.tile([1, 1], F32)
        nc.vector.memset(tiny, 0.0)
        warm = const.tile([1, 1], F32)
        nc.scalar.activation(warm, tiny, GELU)

        # ---- PSUM: two independent 1536-col ring slots (separate tiles so
        # Tile's WAR tracking stays per-slot) + 128x1024 output strip ----
        slotA = ringp.tile([128, 1536], F32, name="slotA")
        slotB = ringp.tile([128, 1536], F32, name="slotB")
        slots = (slotA, slotB)
        strip = stripp.tile([128, 1024], F32)

        # ---- b, bb (y-scale), and ACT/neg variants ----
        tpb = slotB[:, 0:SH]
        nc.tensor.matmul(tpb, w1bT_sb, z2T_sb)
        bb_sb = const.tile([128, SH], F32)
        nc.vector.tensor_scalar_add(bb_sb, tpb, b1y_sb[:, 0:1])
        negbb_sb = const.tile([128, SH], F32)
        nc.vector.tensor_scalar_mul(negbb_sb, bb_sb, -1.0)
        bbx_sb = const.tile([128, SH], F32)
        nc.vector.tensor_scalar_mul(bbx_sb, bb_sb, 1.0 / S_Y)

        # ---- a (y-scale) for all j; fp16 copy for gelu1 inputs ----
        for c in range(8):
            nc.tensor.matmul(
                slotA[:, c * 128:(c + 1) * 128],
                w1aT_sb,
                z1T_sb[:, c * 128:(c + 1) * 128],
            )
        # cast on the (startup-idle) ACT engine so the DVE can go straight
        # into gelu1 custom ops
        a_sb = const.tile([128, N], FP16)
        nc.scalar.activation(a_sb, slotA[:, 0:N],
                             mybir.ActivationFunctionType.Copy)

        out_sb = const.tile([128, N], F32)

        # ---- gelu1 block emission: 16 i-rows per block ----
        NBLK = SH // 16
        h1ts = [None] * NBLK

        def emit_block(b):
            h1t = h1p.tile([128, 16 * N], FP16, tag="h1b", name="h1b", bufs=4)
            for r in range(16):
                il = 16 * b + r
                dst = h1t[:, r * N:(r + 1) * N]
                if _is_act_row(il):
                    nc.scalar.activation(
                        dst, a_sb, GELU,
                        bias=bbx_sb[:, il:il + 1], scale=1.0 / S_Y,
                    )
                else:
                    nc.vector._custom_dve(
                        GELU_OP, out=dst, in0=a_sb[:, 0:N],
                        s0=bb_sb[:, il:il + 1], s1=negbb_sb[:, il:il + 1],
                        imm2=C_T,
                    )
            h1ts[b] = h1t

        emit_block(0)
        emit_block(1)

        # ---- steady state over 1536-wide ring slots ----
        # stream block m (512 cols): i = m//2, j-half = m%2. Slot t covers
        # m in [3t, 3t+3). gelu2 per slot; W3 single-col matmuls accumulate
        # output rows into the strip; 4 chunked evacuations (+b3) + DMAs.
        M_TOT = 2 * SH              # 256 stream blocks
        T_SLOTS = (M_TOT + 2) // 3  # 86
        dma_engs = (nc.sync, nc.gpsimd)
        h2ts = [None] * T_SLOTS

        def emit_evac():
            # any strip read waits on every W3 writer (column-granular dep
            # tracking), so do the whole +b3 evacuation as one op at the end
            nc.vector.tensor_scalar_add(out_sb, strip, b3c_sb[:, 0:1])
            for k in range(8):
                r0 = 16 * k
                dma_engs[k % 2].dma_start(
                    out=out_d.ap()[r0:r0 + 16, :],
                    in_=out_sb[r0:r0 + 16, :],
                )

        def emit_w3(s):
            # W3 single-col matmuls of slot s (emitted 2 slots late so the
            # tensor FIFO never blocks the next W2 fills behind gelu2)
            h2t = h2ts[s]
            for q, m in enumerate(range(3 * s, min(3 * s + 3, M_TOT))):
                i = m // 2
                rg = i // 32
                v = i % 32
                half = m % 2
                nc.tensor.matmul(
                    strip[32 * rg:32 * rg + 32, half * 512:(half + 1) * 512],
                    w3bank_sb[:, 32 * v:32 * v + 32],
                    h2t[:, q * 512:(q + 1) * 512],
                    start=(v == 0),
                    stop=(v == 31),
                    tile_position=(0, 32 * rg),
                    skip_group_check=True,
                )

        for t in range(T_SLOTS):
            ms = range(3 * t, min(3 * t + 3, M_TOT))
            width = 512 * len(ms)
            slot = slots[t % 2]

            # keep gelu1 production two blocks ahead of consumption
            bneed = min((3 * t) // 32 + 2, NBLK - 1)
            while h1ts[bneed] is None:
                nb = next(i for i, v in enumerate(h1ts) if v is None)
                emit_block(nb)

            for q, m in enumerate(ms):
                i = m // 2
                off = (i % 16) * N + (m % 2) * 512
                w2sel = w2T_sb if _is_act_row(i) else w2Td_sb
                nc.tensor.matmul(
                    slot[:, q * 512:(q + 1) * 512],
                    w2sel,
                    h1ts[i // 16][:, off:off + 512],
                )

            h2t = h2p.tile([128, 1536], FP16, tag="h2", bufs=4)
            nc.scalar.activation(
                h2t[:, 0:width], slot[:, 0:width], GELU,
                bias=b2c_sb[:, 0:1],
            )
            h2ts[t] = h2t

            if t >= 2:
                emit_w3(t - 2)

        emit_w3(T_SLOTS - 2)
        emit_w3(T_SLOTS - 1)
        emit_evac()


_NC_CACHE = None


def make_in_maps(z1, z2, W1, b1, W2, b2, W3, b3):
    f = np.float32
    z1 = np.asarray(z1, dtype=f)
    z2 = np.asarray(z2, dtype=f)
    W1 = np.asarray(W1, dtype=f)
    b1 = np.asarray(b1, dtype=f)
    W2 = np.asarray(W2, dtype=f)
    b2 = np.asarray(b2, dtype=f)
    W3 = np.asarray(W3, dtype=f)
    b3 = np.asarray(b3, dtype=f)

    # Host-side relayout/relabel only: transposes, fp16 casts, the constant
    # scale folds described in the module docstring, and the i-shard split.
    z1T = np.ascontiguousarray(z1.T)
    z1c = {
        f"z1c{c}": np.ascontiguousarray(z1T[:, c * 128:(c + 1) * 128])
        for c in range(8)
    }
    w1aT = np.ascontiguousarray((S_Y * W1[:, :EMB]).T)
    w1bT = np.ascontiguousarray((S_Y * W1[:, EMB:]).T)
    b1y = np.ascontiguousarray((S_Y * b1).reshape(HID, 1))
    b2c = np.ascontiguousarray(b2.reshape(HID, 1))
    b3c = np.ascontiguousarray(np.broadcast_to(b3.reshape(1, 1), (HID, 1)))
    w2T = np.ascontiguousarray(W2.T.astype(np.float16))
    w2Td = np.ascontiguousarray((W2.T * WDVE).astype(np.float16))
    w3bank = np.zeros((HID, 32 * 32), dtype=np.float16)
    for v in range(32):
        w3bank[:, 32 * v + v] = W3[0].astype(np.float16)
    w3bank = np.ascontiguousarray(w3bank)

    shared = {
        **z1c, "w1aT": w1aT, "w1bT": w1bT, "b1y": b1y, "b2c": b2c,
        "b3c": b3c, "w2T": w2T, "w2Td": w2Td, "w3bank": w3bank,
    }
    return [
        {**shared, "z2T": np.ascontiguousarray(z2[c * SH:(c + 1) * SH].T)}
        for c in range(NCORES)
    ]


def kernel(z1, z2, W1, b1, W2, b2, W3, b3):
    global _NC_CACHE
    if _NC_CACHE is None:
        _NC_CACHE = _build()
    nc = _NC_CACHE

    in_maps = make_in_maps(z1, z2, W1, b1, W2, b2, W3, b3)
    res = bass_utils.run_bass_kernel_spmd(nc, in_maps, core_ids=list(range(NCORES)))
    return np.concatenate([r["out"] for r in res.results], axis=0)


if __name__ == "__main__":
    rng = np.random.default_rng(0)
    s1 = 1.0 / np.sqrt(2 * EMB)
    s2 = 1.0 / np.sqrt(HID)
    ins = dict(
        z1=rng.standard_normal((N, EMB), dtype=np.float32),
        z2=rng.standard_normal((N, EMB), dtype=np.float32),
        W1=rng.uniform(-s1, s1, (HID, 2 * EMB)).astype(np.float32),
        b1=rng.uniform(-s1, s1, (HID,)).astype(np.float32),
        W2=rng.uniform(-s2, s2, (HID, HID)).astype(np.float32),
        b2=rng.uniform(-s2, s2, (HID,)).astype(np.float32),
        W3=rng.uniform(-s2, s2, (1, HID)).astype(np.float32),
        b3=rng.uniform(-s2, s2, (1,)).astype(np.float32),
    )
    out = kernel(**ins)
    print("out", out.shape, out.dtype, out[:2, :4])


# revision 30
# speedup vs baseline: 1.1964x; 1.1964x over previous
"""Fused pairwise-MLP kernel for Trainium2 (8 NeuronCores, SPMD data-parallel).

Computes log_q[i, j] = W3 @ gelu(W2 @ gelu(a[j] + b[i] + b1) + b2) + b3
with a = z1 @ W1a.T, b = z2 @ W1b.T  (W1 = [W1a | W1b]), N=1024, H=EMB=128.

Sharding: rows of i (z2) split across 8 cores, z1 + weights replicated
(host-side sharding; no collectives). The [N, N, H] intermediates never
touch HBM.

v2: gelu1 is split across TWO engines. Most i-rows run on the DVE via a
custom 8-slice fused op (GELU_QS_ANT): out = xt*(1 + xt*(1 - c*|xt|)) with
xt = a_fp16 + bb_i (the per-i bias add is fused via the per-partition C0/C1
scalar slots; |xt| = max(xt, -xt) built from C1 = -bb_i). In x-space this is
4*q1*x*(0.5 + q1*x - q2*x*|x|), a quadratic-sigmoid gelu approximation
(fitted; end-to-end rel err ~4e-3 << 2e-2 gate). The 1/(4*q1) and lambda
factors fold into a second fp16 W2 copy used only for DVE-produced rows.
The remaining rows run exact gelu on ACT via the bias port (host pre-scales
W1/b1 by s=2*q1; ACT undoes it with scale=1/s). ACT otherwise does gelu2
(1536-wide from the 2x1536 PSUM ring). W3 matmuls accumulate all 128 output
rows into a dedicated 2-bank PSUM strip (row = local i via 32 single-column
W3 variants + tile_position col groups); output leaves via 4 chunked
DVE +b3 evacuations and 8 spread DMAs.
"""

import numpy as np

import concourse.bacc as bacc
import concourse.bass as bass
import concourse.tile as tile
import concourse.mybir as mybir
from concourse import bass_utils
from concourse import dve_ops
from concourse.dve_spec import (
    Spec, Src0, C0, C1, C2, One, maxx, lower, Bin, AluOp,
)
from concourse.dve_uop import DveOpSpec


N = 1024
EMB = 128
HID = 128
NCORES = 8
SH = N // NCORES  # i-rows per core
F32 = mybir.dt.float32
FP16 = mybir.dt.float16
GELU = mybir.ActivationFunctionType.Gelu

# quadratic-sigmoid gelu fit (see docstring): gelu(x) ~= LAM*x*(0.5+Q1*x-Q2*x*|x|)
Q1 = 0.43679
Q2 = 0.09786
LAM = 1.00001
S_Y = 2.0 * Q1                 # host pre-scale of W1/b1
C_T = Q2 / (2.0 * Q1 * Q1)     # |.| coefficient in y-space
WDVE = LAM / (4.0 * Q1)        # folded into the DVE-row W2 copy

# ACT rows per 16-row block (rest go to the DVE custom op)
K_PATTERN = (2, 2, 2, 2, 2, 2, 2, 1)


def _is_act_row(il: int) -> bool:
    return (il % 16) < K_PATTERN[il // 16]


def _register_gelu_op():
    name = "GELU_QS_ANT"
    for op in dve_ops.OPS:
        if op.name == name:
            return op
    xt = Src0 + C0
    neg = Bin(AluOp.SUBTRACT, C1, Src0)      # C1 = -bb  ->  -xt
    u = maxx(xt, neg)                        # |xt|
    w2 = Bin(AluOp.SUBTRACT, One, u * C2)    # 1 - c*|xt|
    body = (w2 * xt + One) * xt              # xt*(1 + xt*(1 - c*|xt|))

    def _ref(in0, in1, s0, s1, imm2):
        x = in0.astype(np.float32) + s0
        return (x * (1.0 + x * (1.0 - imm2 * np.abs(x)))).astype(np.float32)

    spec = Spec(body=body, reference=_ref)
    row = dve_ops._CUSTOM_DVE_ROW_BASE + len(dve_ops.OPS)
    assert row < 0x20
    shas = {}
    for ver in ("v3", "v4"):
        s = DveOpSpec(name=name, opcode=row, uops=lower(spec, ver=ver), rd1_en=False)
        shas[ver] = s.sha(ver)
    op = dve_ops.DveOp(name, spec, subdim=False, uops_sha=shas)
    dve_ops.OPS.append(op)
    dve_ops._SUB_OPCODE_FOR_NAME[name] = row
    dve_ops.CUSTOM_DVE_SPECS[name] = spec
    return op


GELU_OP = _register_gelu_op()


def _build():
    nc = bacc.Bacc("TRN2", target_bir_lowering=False, debug=False)

    z1c_d = [
        nc.dram_tensor(f"z1c{c}", (EMB, 128), F32, kind="ExternalInput")
        for c in range(8)
    ]
    z2T_d = nc.dram_tensor("z2T", (EMB, SH), F32, kind="ExternalInput")
    w1aT_d = nc.dram_tensor("w1aT", (EMB, HID), F32, kind="ExternalInput")
    w1bT_d = nc.dram_tensor("w1bT", (EMB, HID), F32, kind="ExternalInput")
    b1y_d = nc.dram_tensor("b1y", (HID, 1), F32, kind="ExternalInput")
    b2c_d = nc.dram_tensor("b2c", (HID, 1), F32, kind="ExternalInput")
    b3c_d = nc.dram_tensor("b3c", (HID, 1), F32, kind="ExternalInput")
    w2T_d = nc.dram_tensor("w2T", (HID, HID), FP16, kind="ExternalInput")
    w2Td_d = nc.dram_tensor("w2Td", (HID, HID), FP16, kind="ExternalInput")
    w3bank_d = nc.dram_tensor("w3bank", (HID, 32 * 32), FP16, kind="ExternalInput")
    out_d = nc.dram_tensor("out", (SH, N), F32, kind="ExternalOutput")

    with tile.TileContext(nc) as tc:
        _body(tc, out_d, z1c_d, z2T_d, w1aT_d, w1bT_d, b1y_d, b2c_d, b3c_d,
              w2T_d, w2Td_d, w3bank_d)

    nc.compile()
    return nc


def _body(tc, out_d, z1c_d, z2T_d, w1aT_d, w1bT_d, b1y_d, b2c_d, b3c_d,
          w2T_d, w2Td_d, w3bank_d):
    nc = tc.nc
    with (
        tc.tile_pool(name="const", bufs=1) as const,
        tc.tile_pool(name="h1p", bufs=4) as h1p,
        tc.tile_pool(name="h2p", bufs=6) as h2p,
        tc.tile_pool(name="ringp", bufs=1, space="PSUM") as ringp,
        tc.tile_pool(name="stripp", bufs=1, space="PSUM") as stripp,
    ):
        # ---- input DMAs, spread across the engine HWDGE queues ----
        w1aT_sb = const.tile([128, HID], F32)
        nc.scalar.dma_start(out=w1aT_sb, in_=w1aT_d.ap())

        z1T_sb = const.tile([128, N], F32)
        for c, eng in enumerate((nc.sync, nc.scalar, nc.gpsimd, nc.sync,
                                 nc.scalar, nc.gpsimd, nc.sync, nc.scalar)):
            eng.dma_start(out=z1T_sb[:, c * 128:(c + 1) * 128], in_=z1c_d[c].ap())

        z2T_sb = const.tile([128, SH], F32)
        nc.sync.dma_start(out=z2T_sb, in_=z2T_d.ap())
        w1bT_sb = const.tile([128, HID], F32)
        nc.sync.dma_start(out=w1bT_sb, in_=w1bT_d.ap())

        b1y_sb = const.tile([128, 1], F32)
        nc.gpsimd.dma_start(out=b1y_sb, in_=b1y_d.ap())
        b2c_sb = const.tile([128, 1], F32)
        nc.gpsimd.dma_start(out=b2c_sb, in_=b2c_d.ap())
        b3c_sb = const.tile([128, 1], F32)
        nc.gpsimd.dma_start(out=b3c_sb, in_=b3c_d.ap())
        w2T_sb = const.tile([128, HID], FP16)
        nc.gpsimd.dma_start(out=w2T_sb, in_=w2T_d.ap())
        w2Td_sb = const.tile([128, HID], FP16)
        nc.gpsimd.dma_start(out=w2Td_sb, in_=w2Td_d.ap())
        w3bank_sb = const.tile([128, 32 * 32], FP16)
        nc.gpsimd.dma_start(out=w3bank_sb, in_=w3bank_d.ap())

        # warm the gelu ACT table off the critical path
        tiny = const.tile([1, 1], F32)
        nc.vector.memset(tiny, 0.0)
        warm = const.tile([1, 1], F32)
        nc.scalar.activation(warm, tiny, GELU)

        # ---- PSUM: two independent 1536-col ring slots (separate tiles so
        # Tile's WAR tracking stays per-slot) + 128x1024 output strip ----
        slotA = ringp.tile([128, 1536], F32, name="slotA")
        slotB = ringp.tile([128, 1536], F32, name="slotB")
        slots = (slotA, slotB)
        strip = stripp.tile([128, 1024], F32)

        # ---- b, bb (y-scale), and ACT/neg variants ----
        tpb = slotB[:, 0:SH]
        nc.tensor.matmul(tpb, w1bT_sb, z2T_sb)
        bb_sb = const.tile([128, SH], F32)
        nc.vector.tensor_scalar_add(bb_sb, tpb, b1y_sb[:, 0:1])
        negbb_sb = const.tile([128, SH], F32)
        nc.vector.tensor_scalar_mul(negbb_sb, bb_sb, -1.0)
        bbx_sb = const.tile([128, SH], F32)
        nc.vector.tensor_scalar_mul(bbx_sb, bb_sb, 1.0 / S_Y)

        # ---- a (y-scale) for all j; fp16 copy for gelu1 inputs ----
        for c in range(8):
            nc.tensor.matmul(
                slotA[:, c * 128:(c + 1) * 128],
                w1aT_sb,
                z1T_sb[:, c * 128:(c + 1) * 128],
            )
        a_sb = const.tile([128, N], FP16)
        nc.vector.tensor_copy(a_sb, slotA[:, 0:N])

        out_sb = const.tile([128, N], F32)

        # ---- gelu1 block emission: 16 i-rows per block ----
        NBLK = SH // 16
        h1ts = [None] * NBLK

        def emit_block(b):
            h1t = h1p.tile([128, 16 * N], FP16, tag="h1b", name="h1b", bufs=4)
            for r in range(16):
                il = 16 * b + r
                dst = h1t[:, r * N:(r + 1) * N]
                if _is_act_row(il):
                    # first blocks: read a straight from PSUM (no cast dep)
                    a_src = slotA[:, 0:N] if b < 2 else a_sb
                    nc.scalar.activation(
                        dst, a_src, GELU,
                        bias=bbx_sb[:, il:il + 1], scale=1.0 / S_Y,
                    )
                else:
                    nc.vector._custom_dve(
                        GELU_OP, out=dst, in0=a_sb[:, 0:N],
                        s0=bb_sb[:, il:il + 1], s1=negbb_sb[:, il:il + 1],
                        imm2=C_T,
                    )
            h1ts[b] = h1t

        emit_block(0)
        emit_block(1)

        # ---- steady state over 1536-wide ring slots ----
        # stream block m (512 cols): i = m//2, j-half = m%2. Slot t covers
        # m in [3t, 3t+3). gelu2 per slot; W3 single-col matmuls accumulate
        # output rows into the strip; 4 chunked evacuations (+b3) + DMAs.
        M_TOT = 2 * SH              # 256 stream blocks
        T_SLOTS = (M_TOT + 2) // 3  # 86
        dma_engs = (nc.sync, nc.gpsimd)
        h2ts = [None] * T_SLOTS

        def emit_evac():
            # any strip read waits on every W3 writer (column-granular dep
            # tracking), so do the whole +b3 evacuation as one op at the end
            nc.vector.tensor_scalar_add(out_sb, strip, b3c_sb[:, 0:1])
            for k in range(8):
                r0 = 16 * k
                # scalar engine is idle by now; avoids extra queue drains
                (nc.sync, nc.scalar)[k % 2].dma_start(
                    out=out_d.ap()[r0:r0 + 16, :],
                    in_=out_sb[r0:r0 + 16, :],
                )

        def emit_w3(s):
            # W3 single-col matmuls of slot s (emitted 2 slots late so the
            # tensor FIFO never blocks the next W2 fills behind gelu2)
            h2t = h2ts[s]
            for q, m in enumerate(range(3 * s, min(3 * s + 3, M_TOT))):
                i = m // 2
                rg = i // 32
                v = i % 32
                half = m % 2
                nc.tensor.matmul(
                    strip[32 * rg:32 * rg + 32, half * 512:(half + 1) * 512],
                    w3bank_sb[:, 32 * v:32 * v + 32],
                    h2t[:, q * 512:(q + 1) * 512],
                    start=(v == 0),
                    stop=(v == 31),
                    tile_position=(0, 32 * rg),
                    skip_group_check=True,
                )

        for t in range(T_SLOTS):
            ms = range(3 * t, min(3 * t + 3, M_TOT))
            width = 512 * len(ms)
            slot = slots[t % 2]

            # keep gelu1 production two blocks ahead of consumption
            bneed = min((3 * t) // 32 + 2, NBLK - 1)
            while h1ts[bneed] is None:
                nb = next(i for i, v in enumerate(h1ts) if v is None)
                emit_block(nb)

            for q, m in enumerate(ms):
                i = m // 2
                off = (i % 16) * N + (m % 2) * 512
                w2sel = w2T_sb if _is_act_row(i) else w2Td_sb
                nc.tensor.matmul(
                    slot[:, q * 512:(q + 1) * 512],
                    w2sel,
                    h1ts[i // 16][:, off:off + 512],
                )

            h2t = h2p.tile([128, 1536], FP16, tag="h2", bufs=4)
            nc.scalar.activation(
                h2t[:, 0:width], slot[:, 0:width], GELU,
                bias=b2c_sb[:, 0:1],
            )
            h2ts[t] = h2t

            if t >= 2:
                emit_w3(t - 2)

        emit_w3(T_SLOTS - 2)
        emit_w3(T_SLOTS - 1)
        emit_evac()


_NC_CACHE = None


def make_in_maps(z1, z2, W1, b1, W2, b2, W3, b3):
    f = np.float32
    z1 = np.asarray(z1, dtype=f)
    z2 = np.asarray(z2, dtype=f)
    W1 = np.asarray(W1, dtype=f)
    b1 = np.asarray(b1, dtype=f)
    W2 = np.asarray(W2, dtype=f)
    b2 = np.asarray(b2, dtype=f)
    W3 = np.asarray(W3, dtype=f)
    b3 = np.asarray(b3, dtype=f)

    # Host-side relayout/relabel only: transposes, fp16 casts, the constant
    # scale folds described in the module docstring, and the i-shard split.
    z1T = np.ascontiguousarray(z1.T)
    z1c = {
        f"z1c{c}": np.ascontiguousarray(z1T[:, c * 128:(c + 1) * 128])
        for c in range(8)
    }
    w1aT = np.ascontiguousarray((S_Y * W1[:, :EMB]).T)
    w1bT = np.ascontiguousarray((S_Y * W1[:, EMB:]).T)
    b1y = np.ascontiguousarray((S_Y * b1).reshape(HID, 1))
    b2c = np.ascontiguousarray(b2.reshape(HID, 1))
    b3c = np.ascontiguousarray(np.broadcast_to(b3.reshape(1, 1), (HID, 1)))
    w2T = np.ascontiguousarray(W2.T.astype(np.float16))
    w2Td = np.ascontiguousarray((W2.T * WDVE).astype(np.float16))
    w3bank = np.zeros((HID, 32 * 32), dtype=np.float16)
    for v in range(32):
        w3bank[:, 32 * v + v] = W3[0].astype(np.float16)
    w3bank = np.ascontiguousarray(w3bank)

    shared = {
        **z1c, "w1aT": w1aT, "w1bT": w1bT, "b1y": b1y, "b2c": b2c,
        "b3c": b3c, "w2T": w2T, "w2Td": w2Td, "w3bank": w3bank,
    }
    return [
        {**shared, "z2T": np.ascontiguousarray(z2[c * SH:(c + 1) * SH].T)}
        for c in range(NCORES)
    ]


def kernel(z1, z2, W1, b1, W2, b2, W3, b3):
    global _NC_CACHE
    if _NC_CACHE is None:
        _NC_CACHE = _build()
    nc = _NC_CACHE

    in_maps = make_in_maps(z1, z2, W1, b1, W2, b2, W3, b3)
    res = bass_utils.run_bass_kernel_spmd(nc, in_maps, core_ids=list(range(NCORES)))
    return np.concatenate([r["out"] for r in res.results], axis=0)


if __name__ == "__main__":
    rng = np.random.default_rng(0)
    s1 = 1.0 / np.sqrt(2 * EMB)
    s2 = 1.0 / np.sqrt(HID)
    ins = dict(
        z1=rng.standard_normal((N, EMB), dtype=np.float32),
        z2=rng.standard_normal((N, EMB), dtype=np.float32),
        W1=rng.uniform(-s1, s1, (HID, 2 * EMB)).astype(np.float32),
        b1=rng.uniform(-s1, s1, (HID,)).astype(np.float32),
        W2=rng.uniform(-s2, s2, (HID, HID)).astype(np.float32),
        b2=rng.uniform(-s2, s2, (HID,)).astype(np.float32),
        W3=rng.uniform(-s2, s2, (1, HID)).astype(np.float32),
        b3=rng.uniform(-s2, s2, (1,)).astype(np.float32),
    )
    out = kernel(**ins)
    print("out", out.shape, out.dtype, out[:2, :4])


# revision 32
# speedup vs baseline: 1.1997x; 1.0027x over previous
"""Fused pairwise-MLP kernel for Trainium2 (8 NeuronCores, SPMD data-parallel).

Computes log_q[i, j] = W3 @ gelu(W2 @ gelu(a[j] + b[i] + b1) + b2) + b3
with a = z1 @ W1a.T, b = z2 @ W1b.T  (W1 = [W1a | W1b]), N=1024, H=EMB=128.

Sharding: rows of i (z2) split across 8 cores, z1 + weights replicated
(host-side sharding; no collectives). The [N, N, H] intermediates never
touch HBM.

v2: gelu1 is split across TWO engines. Most i-rows run on the DVE via a
custom 8-slice fused op (GELU_QS_ANT): out = xt*(1 + xt*(1 - c*|xt|)) with
xt = a_fp16 + bb_i (the per-i bias add is fused via the per-partition C0/C1
scalar slots; |xt| = max(xt, -xt) built from C1 = -bb_i). In x-space this is
4*q1*x*(0.5 + q1*x - q2*x*|x|), a quadratic-sigmoid gelu approximation
(fitted; end-to-end rel err ~4e-3 << 2e-2 gate). The 1/(4*q1) and lambda
factors fold into a second fp16 W2 copy used only for DVE-produced rows.
The remaining rows run exact gelu on ACT via the bias port (host pre-scales
W1/b1 by s=2*q1; ACT undoes it with scale=1/s). ACT otherwise does gelu2
(1536-wide from the 2x1536 PSUM ring). W3 matmuls accumulate all 128 output
rows into a dedicated 2-bank PSUM strip (row = local i via 32 single-column
W3 variants + tile_position col groups); output leaves via 4 chunked
DVE +b3 evacuations and 8 spread DMAs.
"""

import numpy as np

import concourse.bacc as bacc
import concourse.bass as bass
import concourse.tile as tile
import concourse.mybir as mybir
from concourse import bass_utils
from concourse import dve_ops
from concourse.dve_spec import (
    Spec, Src0, C0, C1, C2, One, maxx, lower, Bin, AluOp,
)
from concourse.dve_uop import DveOpSpec


N = 1024
EMB = 128
HID = 128
NCORES = 8
SH = N // NCORES  # i-rows per core
F32 = mybir.dt.float32
FP16 = mybir.dt.float16
GELU = mybir.ActivationFunctionType.Gelu

# quadratic-sigmoid gelu fit (see docstring): gelu(x) ~= LAM*x*(0.5+Q1*x-Q2*x*|x|)
Q1 = 0.43679
Q2 = 0.09786
LAM = 1.00001
S_Y = 2.0 * Q1                 # host pre-scale of W1/b1
C_T = Q2 / (2.0 * Q1 * Q1)     # |.| coefficient in y-space
WDVE = LAM / (4.0 * Q1)        # folded into the DVE-row W2 copy

# ACT rows per 16-row block (rest go to the DVE custom op)
K_PATTERN = (2, 2, 2, 2, 2, 2, 2, 1)


def _is_act_row(il: int) -> bool:
    return (il % 16) < K_PATTERN[il // 16]


def _register_gelu_op():
    name = "GELU_QS_ANT"
    for op in dve_ops.OPS:
        if op.name == name:
            return op
    xt = Src0 + C0
    neg = Bin(AluOp.SUBTRACT, C1, Src0)      # C1 = -bb  ->  -xt
    u = maxx(xt, neg)                        # |xt|
    w2 = Bin(AluOp.SUBTRACT, One, u * C2)    # 1 - c*|xt|
    body = (w2 * xt + One) * xt              # xt*(1 + xt*(1 - c*|xt|))

    def _ref(in0, in1, s0, s1, imm2):
        x = in0.astype(np.float32) + s0
        return (x * (1.0 + x * (1.0 - imm2 * np.abs(x)))).astype(np.float32)

    spec = Spec(body=body, reference=_ref)
    row = dve_ops._CUSTOM_DVE_ROW_BASE + len(dve_ops.OPS)
    assert row < 0x20
    shas = {}
    for ver in ("v3", "v4"):
        s = DveOpSpec(name=name, opcode=row, uops=lower(spec, ver=ver), rd1_en=False)
        shas[ver] = s.sha(ver)
    op = dve_ops.DveOp(name, spec, subdim=False, uops_sha=shas)
    dve_ops.OPS.append(op)
    dve_ops._SUB_OPCODE_FOR_NAME[name] = row
    dve_ops.CUSTOM_DVE_SPECS[name] = spec
    return op


GELU_OP = _register_gelu_op()


def _build():
    nc = bacc.Bacc("TRN2", target_bir_lowering=False, debug=False)

    z1c_d = [
        nc.dram_tensor(f"z1c{c}", (EMB, 128), F32, kind="ExternalInput")
        for c in range(8)
    ]
    z2T_d = nc.dram_tensor("z2T", (EMB, SH), F32, kind="ExternalInput")
    w1aT_d = nc.dram_tensor("w1aT", (EMB, HID), F32, kind="ExternalInput")
    w1bT_d = nc.dram_tensor("w1bT", (EMB, HID), F32, kind="ExternalInput")
    b1y_d = nc.dram_tensor("b1y", (HID, 1), F32, kind="ExternalInput")
    b2c_d = nc.dram_tensor("b2c", (HID, 1), F32, kind="ExternalInput")
    b3c_d = nc.dram_tensor("b3c", (HID, 1), F32, kind="ExternalInput")
    w2T_d = nc.dram_tensor("w2T", (HID, HID), FP16, kind="ExternalInput")
    w2Td_d = nc.dram_tensor("w2Td", (HID, HID), FP16, kind="ExternalInput")
    w3bank_d = nc.dram_tensor("w3bank", (HID, 32 * 32), FP16, kind="ExternalInput")
    out_d = nc.dram_tensor("out", (SH, N), F32, kind="ExternalOutput")

    with tile.TileContext(nc) as tc:
        _body(tc, out_d, z1c_d, z2T_d, w1aT_d, w1bT_d, b1y_d, b2c_d, b3c_d,
              w2T_d, w2Td_d, w3bank_d)

    nc.compile()
    return nc


def _body(tc, out_d, z1c_d, z2T_d, w1aT_d, w1bT_d, b1y_d, b2c_d, b3c_d,
          w2T_d, w2Td_d, w3bank_d):
    nc = tc.nc
    with (
        tc.tile_pool(name="const", bufs=1) as const,
        tc.tile_pool(name="h1p", bufs=4) as h1p,
        tc.tile_pool(name="h2p", bufs=6) as h2p,
        tc.tile_pool(name="ringp", bufs=1, space="PSUM") as ringp,
        tc.tile_pool(name="stripp", bufs=1, space="PSUM") as stripp,
    ):
        # ---- input DMAs, spread across the engine HWDGE queues ----
        w1aT_sb = const.tile([128, HID], F32)
        nc.scalar.dma_start(out=w1aT_sb, in_=w1aT_d.ap())
        z2T_sb = const.tile([128, SH], F32)
        nc.gpsimd.dma_start(out=z2T_sb, in_=z2T_d.ap())
        w1bT_sb = const.tile([128, HID], F32)
        nc.gpsimd.dma_start(out=w1bT_sb, in_=w1bT_d.ap())
        b1y_sb = const.tile([128, 1], F32)
        nc.gpsimd.dma_start(out=b1y_sb, in_=b1y_d.ap())

        z1T_sb = const.tile([128, N], F32)
        for c, eng in enumerate((nc.sync, nc.scalar, nc.sync, nc.scalar,
                                 nc.sync, nc.scalar, nc.sync, nc.scalar)):
            eng.dma_start(out=z1T_sb[:, c * 128:(c + 1) * 128], in_=z1c_d[c].ap())

        b2c_sb = const.tile([128, 1], F32)
        nc.gpsimd.dma_start(out=b2c_sb, in_=b2c_d.ap())
        b3c_sb = const.tile([128, 1], F32)
        nc.gpsimd.dma_start(out=b3c_sb, in_=b3c_d.ap())
        w2T_sb = const.tile([128, HID], FP16)
        nc.gpsimd.dma_start(out=w2T_sb, in_=w2T_d.ap())
        w2Td_sb = const.tile([128, HID], FP16)
        nc.gpsimd.dma_start(out=w2Td_sb, in_=w2Td_d.ap())
        w3bank_sb = const.tile([128, 32 * 32], FP16)
        nc.gpsimd.dma_start(out=w3bank_sb, in_=w3bank_d.ap())

        # warm the gelu ACT table off the critical path (memset on gpsimd so
        # the warm-up doesn't wait on the DVE queue)
        tiny = const.tile([1, 1], F32)
        nc.gpsimd.memset(tiny, 0.0)
        warm = const.tile([1, 1], F32)
        nc.scalar.activation(warm, tiny, GELU)

        # ---- PSUM: two independent 1536-col ring slots (separate tiles so
        # Tile's WAR tracking stays per-slot) + 128x1024 output strip ----
        slotA = ringp.tile([128, 1536], F32, name="slotA")
        slotB = ringp.tile([128, 1536], F32, name="slotB")
        slots = (slotA, slotB)
        strip = stripp.tile([128, 1024], F32)

        # ---- b, bb (y-scale), and ACT/neg variants ----
        tpb = slotB[:, 0:SH]
        nc.tensor.matmul(tpb, w1bT_sb, z2T_sb)
        bb_sb = const.tile([128, SH], F32)
        nc.vector.tensor_scalar_add(bb_sb, tpb, b1y_sb[:, 0:1])
        negbb_sb = const.tile([128, SH], F32)
        nc.vector.tensor_scalar_mul(negbb_sb, bb_sb, -1.0)
        bbx_sb = const.tile([128, SH], F32)
        nc.vector.tensor_scalar_mul(bbx_sb, bb_sb, 1.0 / S_Y)

        # ---- a (y-scale) for all j; fp16 copy for gelu1 inputs ----
        for c in range(8):
            nc.tensor.matmul(
                slotA[:, c * 128:(c + 1) * 128],
                w1aT_sb,
                z1T_sb[:, c * 128:(c + 1) * 128],
            )
        a_sb = const.tile([128, N], FP16)
        nc.vector.tensor_copy(a_sb, slotA[:, 0:N])

        out_sb = const.tile([128, N], F32)

        # ---- gelu1 block emission: 16 i-rows per block ----
        NBLK = SH // 16
        h1ts = [None] * NBLK

        def emit_block(b):
            h1t = h1p.tile([128, 16 * N], FP16, tag="h1b", name="h1b", bufs=4)
            for r in range(16):
                il = 16 * b + r
                dst = h1t[:, r * N:(r + 1) * N]
                if _is_act_row(il):
                    # first blocks: read a straight from PSUM (no cast dep)
                    a_src = slotA[:, 0:N] if b < 2 else a_sb
                    nc.scalar.activation(
                        dst, a_src, GELU,
                        bias=bbx_sb[:, il:il + 1], scale=1.0 / S_Y,
                    )
                else:
                    nc.vector._custom_dve(
                        GELU_OP, out=dst, in0=a_sb[:, 0:N],
                        s0=bb_sb[:, il:il + 1], s1=negbb_sb[:, il:il + 1],
                        imm2=C_T,
                    )
            h1ts[b] = h1t

        emit_block(0)
        emit_block(1)

        # ---- steady state over 1536-wide ring slots ----
        # stream block m (512 cols): i = m//2, j-half = m%2. Slot t covers
        # m in [3t, 3t+3). gelu2 per slot; W3 single-col matmuls accumulate
        # output rows into the strip; 4 chunked evacuations (+b3) + DMAs.
        M_TOT = 2 * SH              # 256 stream blocks
        T_SLOTS = (M_TOT + 2) // 3  # 86
        dma_engs = (nc.sync, nc.gpsimd)
        h2ts = [None] * T_SLOTS

        def emit_evac():
            # any strip read waits on every W3 writer (column-granular dep
            # tracking), so do the whole +b3 evacuation as one op at the end
            nc.vector.tensor_scalar_add(out_sb, strip, b3c_sb[:, 0:1])
            for k in range(8):
                r0 = 16 * k
                # scalar engine is idle by now; avoids extra queue drains
                (nc.sync, nc.scalar)[k % 2].dma_start(
                    out=out_d.ap()[r0:r0 + 16, :],
                    in_=out_sb[r0:r0 + 16, :],
                )

        def emit_w3(s):
            # W3 single-col matmuls of slot s (emitted 2 slots late so the
            # tensor FIFO never blocks the next W2 fills behind gelu2)
            h2t = h2ts[s]
            for q, m in enumerate(range(3 * s, min(3 * s + 3, M_TOT))):
                i = m // 2
                rg = i // 32
                v = i % 32
                half = m % 2
                nc.tensor.matmul(
                    strip[32 * rg:32 * rg + 32, half * 512:(half + 1) * 512],
                    w3bank_sb[:, 32 * v:32 * v + 32],
                    h2t[:, q * 512:(q + 1) * 512],
                    start=(v == 0),
                    stop=(v == 31),
                    tile_position=(0, 32 * rg),
                    skip_group_check=True,
                )

        for t in range(T_SLOTS):
            ms = range(3 * t, min(3 * t + 3, M_TOT))
            width = 512 * len(ms)
            slot = slots[t % 2]

            # keep gelu1 production two blocks ahead of consumption
            bneed = min((3 * t) // 32 + 2, NBLK - 1)
            while h1ts[bneed] is None:
                nb = next(i for i, v in enumerate(h1ts) if v is None)
                emit_block(nb)

            for q, m in enumerate(ms):
                i = m // 2
                off = (i % 16) * N + (m % 2) * 512
                w2sel = w2T_sb if _is_act_row(i) else w2Td_sb
                nc.tensor.matmul(
                    slot[:, q * 512:(q + 1) * 512],
                    w2sel,
                    h1ts[i // 16][:, off:off + 512],
                )

            h2t = h2p.tile([128, 1536], FP16, tag="h2", bufs=4)
            nc.scalar.activation(
                h2t[:, 0:width], slot[:, 0:width], GELU,
                bias=b2c_sb[:, 0:1],
            )
            h2ts[t] = h2t

            if t >= 2:
                emit_w3(t - 2)

        emit_w3(T_SLOTS - 2)
        emit_w3(T_SLOTS - 1)
        emit_evac()


_NC_CACHE = None


def make_in_maps(z1, z2, W1, b1, W2, b2, W3, b3):
    f = np.float32
    z1 = np.asarray(z1, dtype=f)
    z2 = np.asarray(z2, dtype=f)
    W1 = np.asarray(W1, dtype=f)
    b1 = np.asarray(b1, dtype=f)
    W2 = np.asarray(W2, dtype=f)
    b2 = np.asarray(b2, dtype=f)
    W3 = np.asarray(W3, dtype=f)
    b3 = np.asarray(b3, dtype=f)

    # Host-side relayout/relabel only: transposes, fp16 casts, the constant
    # scale folds described in the module docstring, and the i-shard split.
    z1T = np.ascontiguousarray(z1.T)
    z1c = {
        f"z1c{c}": np.ascontiguousarray(z1T[:, c * 128:(c + 1) * 128])
        for c in range(8)
    }
    w1aT = np.ascontiguousarray((S_Y * W1[:, :EMB]).T)
    w1bT = np.ascontiguousarray((S_Y * W1[:, EMB:]).T)
    b1y = np.ascontiguousarray((S_Y * b1).reshape(HID, 1))
    b2c = np.ascontiguousarray(b2.reshape(HID, 1))
    b3c = np.ascontiguousarray(np.broadcast_to(b3.reshape(1, 1), (HID, 1)))
    w2T = np.ascontiguousarray(W2.T.astype(np.float16))
    w2Td = np.ascontiguousarray((W2.T * WDVE).astype(np.float16))
    w3bank = np.zeros((HID, 32 * 32), dtype=np.float16)
    for v in range(32):
        w3bank[:, 32 * v + v] = W3[0].astype(np.float16)
    w3bank = np.ascontiguousarray(w3bank)

    shared = {
        **z1c, "w1aT": w1aT, "w1bT": w1bT, "b1y": b1y, "b2c": b2c,
        "b3c": b3c, "w2T": w2T, "w2Td": w2Td, "w3bank": w3bank,
    }
    return [
        {**shared, "z2T": np.ascontiguousarray(z2[c * SH:(c + 1) * SH].T)}
        for c in range(NCORES)
    ]


def kernel(z1, z2, W1, b1, W2, b2, W3, b3):
    global _NC_CACHE
    if _NC_CACHE is None:
        _NC_CACHE = _build()
    nc = _NC_CACHE

    in_maps = make_in_maps(z1, z2, W1, b1, W2, b2, W3, b3)
    res = bass_utils.run_bass_kernel_spmd(nc, in_maps, core_ids=list(range(NCORES)))
    return np.concatenate([r["out"] for r in res.results], axis=0)


if __name__ == "__main__":
    rng = np.random.default_rng(0)
    s1 = 1.0 / np.sqrt(2 * EMB)
    s2 = 1.0 / np.sqrt(HID)
    ins = dict(
        z1=rng.standard_normal((N, EMB), dtype=np.float32),
        z2=rng.standard_normal((N, EMB), dtype=np.float32),
        W1=rng.uniform(-s1, s1, (HID, 2 * EMB)).astype(np.float32),
        b1=rng.uniform(-s1, s1, (HID,)).astype(np.float32),
        W2=rng.uniform(-s2, s2, (HID, HID)).astype(np.float32),
        b2=rng.uniform(-s2, s2, (HID,)).astype(np.float32),
        W3=rng.uniform(-s2, s2, (1, HID)).astype(np.float32),
        b3=rng.uniform(-s2, s2, (1,)).astype(np.float32),
    )
    out = kernel(**ins)
    print("out", out.shape, out.dtype, out[:2, :4])
